# revision 57
# baseline (speedup 1.0000x reference)
"""Multi-head causal attention (B=4, T=2048, C=768, H=12, HS=64) on 8 trn2 cores.

Sharding: 48 (batch, head) units -> 6 per core. Core c: batch c//2, heads
6*(c%2) .. 6*(c%2)+6. Each core computes a partial output projection
y_partial[T, C] (bf16); host sums the two partials per batch in f32 and adds
the bias.

Per-core layout (heads processed as 3 pairs; pair p = heads (2p, 2p+1)):
  xT      [C, T]          input, pre-transposed on host, bf16
  pairQ/pairK [128, T]    head parity e at partitions 64e..64e+64; produced
                          DIRECTLY by the QK projection with pair-stacked
                          weights (no staging/DMA shuffle). Weights are
                          partition-major in DRAM so each load is ONE DMA
                          (sync-engine DMA dispatch costs ~650ns apiece).
  scores  ST[tk, 2, tq]   matmul(lhsT=pairK rows, rhs=pairQ rows) per head,
                          row-tiled via tile_position=(64e, 0) so the two
                          heads' matmuls overlap in the PE array.
  softmax                 no max-subtraction (scores*scale are O(+-8); exp is
                          safe in f32); row sums via ones-columns in vaug.
  V       vaug[tk, tt, p, 130] bf16: cols 0-63 V_even, 64 ones, 65-128 V_odd,
                          129 ones. PV lhsT e0 = [0:65], e1 = [65:130] so BOTH
                          heads put data at psum rows 0-63 and denom at row 64.
  P@V     OTu[65, 2, tq] accumulated over tk tiles.
  norm    copy OTu->SBUF; the denominators live on one partition, so they are
                          repacked to [128, 8] by a tiny SBUF DMA, recip'd
                          lane-parallel, cast to bf16 and unpacked; a K=1
                          ones-matmul broadcasts to rb[64, tq] and one fused
                          multiply+cast normalizes. Even head writes otn[0:64]
                          in place; odd head DMAs to otn[64:128].
  proj    y[tq, :] = sum_g matmul(lhsT=otn[:, g, tq], rhs=WpT[g])

Scheduling (the real speed): the ScalarE exp stream (~120us) and the PE
matmul stream (~180us) are co-scheduled by EMISSION order since every engine
executes in order. Each pair's P@V matmuls are deferred into the NEXT pair's
scores loop (their inputs are ready there) and every other unit of
projection work (QK chunks, V tiles, previous chunk's output projection) is
kept in a dependency-tracked filler queue drained one unit per score tile —
so the PE never idles long enough for the HAM clock gate to drop it to
1.2 GHz. The per-pair reciprocal chain (copy -> pack DMA -> recip -> cast ->
unpack DMA) is latency-heavy, so its consumer (the broadcast matmul) is
deferred one further pair. A few proj units are reserved for the last pair's
loop and dependency-free "warm-keeper" matmuls bridge the tail stalls.
"""

import numpy as np
import ml_dtypes

import concourse.bacc as bacc
import concourse.bass as bass
import concourse.tile as tile
from concourse import mybir
from concourse import bass_utils

B, T, C = 4, 2048, 768
H, HS = 12, 64
HL = 6            # heads per core
NP = 3            # head pairs per core
NCT = C // 128    # 6 contraction tiles
NTT = T // 128    # 16 t tiles
NTC = T // 512    # 4 t chunks
SCALE = 1.0 / 8.0  # 1/sqrt(HS)

F32 = mybir.dt.float32
BF16 = mybir.dt.bfloat16


def build_kernel(nc, repeat=1, phases=("v", "qk", "attn", "norm", "proj")):
    # weight layouts are partition-major so each load is ONE DMA (the sync
    # engine serializes DMA dispatch at ~650ns each; many small loads would
    # stall the kernel head)
    xT = nc.dram_tensor("xT", [C, T], BF16, kind="ExternalInput").ap()
    wqkp = nc.dram_tensor("wqkp", [NP, 2, 128, NCT, 128], BF16,
                          kind="ExternalInput").ap()
    wv = nc.dram_tensor("wv", [128, NCT, HL * HS], BF16,
                        kind="ExternalInput").ap()
    wpt = nc.dram_tensor("wpt", [128, 3, C], BF16, kind="ExternalInput").ap()
    # bf16 partials: the host sums the two per-batch partials in f32; the
    # ~0.4% partial rounding is well inside the error budget and halves the
    # output copy + DMA cost
    y = nc.dram_tensor("y", [T, C], BF16, kind="ExternalOutput").ap()

    with tile.TileContext(nc) as tc:
        with (
            tc.tile_pool(name="consts", bufs=1) as consts,
            tc.tile_pool(name="xw", bufs=1) as xw,
            tc.tile_pool(name="pt", bufs=32) as ptp,
            tc.tile_pool(name="small", bufs=3) as small,
            tc.tile_pool(name="ysb", bufs=2) as ysbp,
            # PSUM: st (2 slots x 2 banks) + otu (1 slot x 2 banks) +
            # tt (2 slots x 1 bank, shared by psv/psqk/rb/y1/y2) = 8 banks
            tc.tile_pool(name="ps_st", bufs=2, space="PSUM") as ps_st,
            tc.tile_pool(name="ps_otu", bufs=1, space="PSUM") as ps_otu,
            tc.tile_pool(name="ps_t", bufs=2, space="PSUM") as ps_t,
        ):
            # ---------------- weights + x ----------------
            # DMA order matters for the kernel head: pair-0 QK weights land
            # first so the first projection matmuls start ~2us in, paced by
            # the x tiles as they arrive.
            wqkp_t = [[None, None] for _ in range(NP)]

            def load_wqkp(p):
                for qk in range(2):
                    t_ = xw.tile([128, NCT, 128], BF16, tag=f"wqkp{p}_{qk}",
                                 name=f"wqkp{p}_{qk}")
                    nc.sync.dma_start(out=t_, in_=wqkp[p, qk])
                    wqkp_t[p][qk] = t_

            load_wqkp(0)
            xt = []
            for ci in range(NCT):
                t_ = xw.tile([128, T], BF16, tag=f"xt{ci}", name=f"xt{ci}")
                nc.sync.dma_start(out=t_, in_=xT[ci * 128:(ci + 1) * 128, :])
                xt.append(t_)
            wv_t = xw.tile([128, NCT, HL * HS], BF16, tag="wv", name="wv")
            nc.sync.dma_start(out=wv_t, in_=wv)
            load_wqkp(1)
            load_wqkp(2)
            wpt_t = consts.tile([128, 3, C], BF16, tag="wpt", name="wpt")
            nc.sync.dma_start(out=wpt_t, in_=wpt)
            wqkp_sb = [
                [[wqkp_t[p][qk][:, ci, :] for ci in range(NCT)]
                 for qk in range(2)]
                for p in range(NP)
            ]
            wv_sb = [wv_t[:, ci, :] for ci in range(NCT)]
            wpt_sb = [wpt_t[:, g, :] for g in range(3)]

            # persistent tensors (allocated once; loop iterations rewrite)
            vaug = consts.tile([128, NTT, NP, 130], BF16)
            nc.gpsimd.memset(vaug[:, :, :, 64:65], 1.0)
            nc.gpsimd.memset(vaug[:, :, :, 129:130], 1.0)
            pairQ = [consts.tile([128, T], BF16, tag=f"pq{p}", name=f"pq{p}")
                     for p in range(NP)]
            pairK = [consts.tile([128, T], BF16, tag=f"pk{p}", name=f"pk{p}")
                     for p in range(NP)]
            otn = consts.tile([128, NP, T], BF16)
            if "v" not in phases:
                nc.gpsimd.memset(vaug[:, :, :, 0:64], 0.0)
                nc.gpsimd.memset(vaug[:, :, :, 65:129], 0.0)
            if "qk" not in phases:
                for p in range(NP):
                    nc.gpsimd.memset(pairQ[p], 0.0)
                    nc.gpsimd.memset(pairK[p], 0.0)
            if "norm" not in phases:
                nc.gpsimd.memset(otn, 0.0)

            import contextlib
            rep_ctx = (
                tc.For_i(0, repeat, 1,
                         hint_engines=(mybir.EngineType.PE,
                                       mybir.EngineType.DVE,
                                       mybir.EngineType.Activation,
                                       mybir.EngineType.SP,
                                       mybir.EngineType.Pool))
                if repeat > 1 else contextlib.nullcontext()
            )
            with rep_ctx:
                build_phases(nc, tc, consts, xw, ptp, small, ysbp,
                             ps_st, ps_otu, ps_t,
                             xt, wqkp_sb, wv_sb, wpt_sb,
                             vaug, pairQ, pairK, otn, y, phases)

    nc.compile()
    return nc


def build_phases(nc, tc, consts, xw, ptp, small, ysbp,
                 ps_st, ps_otu, ps_t,
                 xt, wqkp_sb, wv_sb, wpt_sb,
                 vaug, pairQ, pairK, otn, y,
                 phases=("v", "qk", "attn", "norm", "proj")):
    ones_rows = consts.tile([128, HS + 1], BF16)
    nc.gpsimd.memset(ones_rows, 1.0)

    do_v = "v" in phases
    do_qk = "qk" in phases
    do_attn = "attn" in phases
    do_norm = "norm" in phases
    do_proj = "proj" in phases

    def emit_vproj(tt):
        # vaug[:, tt, p, 0:64] = V of head 2p, [65:129] = V of head 2p+1
        ps = ps_t.tile([128, HL * HS], F32, tag="tt", name="psv")
        for ci in range(NCT):
            nc.tensor.matmul(
                ps, xt[ci][:, tt * 128:(tt + 1) * 128], wv_sb[ci],
                start=(ci == 0), stop=(ci == NCT - 1),
            )
        nc.vector.tensor_copy(
            out=vaug[:, tt, :, 0:HS],
            in_=ps[:, 0:NP * HS].rearrange("p (g d) -> p g d", g=NP),
        )
        nc.vector.tensor_copy(
            out=vaug[:, tt, :, HS + 1:2 * HS + 1],
            in_=ps[:, NP * HS:2 * NP * HS].rearrange("p (g d) -> p g d", g=NP),
        )

    def emit_qkproj(p, qk, mm):
        # one 512-wide chunk of pairQ[p]/pairK[p] in pair-stacked layout
        dst = pairQ[p] if qk == 0 else pairK[p]
        sl = slice(mm * 512, (mm + 1) * 512)
        ps = ps_t.tile([128, 512], F32, tag="tt", name="psqk")
        for ci in range(NCT):
            nc.tensor.matmul(
                ps, wqkp_sb[p][qk][ci], xt[ci][:, sl],
                start=(ci == 0), stop=(ci == NCT - 1),
            )
        nc.vector.tensor_copy(out=dst[:, sl], in_=ps)

    tail_phase = [False]

    def emit_proj(tt):
        # output projection for one 128-row tq tile
        y1 = ps_t.tile([128, 512], F32, tag="tt", name="y1")
        y2 = ps_t.tile([128, 256], F32, tag="tt", name="y2")
        for g in range(3):
            lhs = otn[:, g, tt * 128:(tt + 1) * 128]
            nc.tensor.matmul(
                y1, lhs, wpt_sb[g][:, 0:512],
                start=(g == 0), stop=(g == 2),
            )
            nc.tensor.matmul(
                y2, lhs, wpt_sb[g][:, 512:768],
                start=(g == 0), stop=(g == 2),
            )
        ysb = ysbp.tile([128, C], BF16, tag="ysb", name="ysb")
        if tail_phase[0]:
            # ACT is idle after the last exp; take the PSUM drain off the
            # (busy) vector queue in the tail
            nc.scalar.copy(out=ysb[:, 0:512], in_=y1)
            nc.scalar.copy(out=ysb[:, 512:768], in_=y2)
        else:
            nc.vector.tensor_copy(out=ysb[:, 0:512], in_=y1)
            nc.vector.tensor_copy(out=ysb[:, 512:768], in_=y2)
        nc.sync.dma_start(out=y[tt * 128:(tt + 1) * 128, :], in_=ysb)

    # ---- filler queue: every unit of projection work (one PSUM group) in
    # a dependency-respecting global order. Units are drained one-at-a-time
    # into the gaps of the ACT-paced scores loops so the PE never idles
    # long enough for the HAM clock gate to re-throttle it; force() emits a
    # unit immediately when a consumer needs it.
    emitted = set()

    def emit_unit(u):
        if u in emitted:
            return
        emitted.add(u)
        kind = u[0]
        if kind == "qk":
            emit_qkproj(u[1], u[2], u[3])
        elif kind == "v":
            emit_vproj(u[1])
        else:
            emit_proj(u[1])

    order = list(range(NTC))
    pos = {mm: mm for mm in order}

    fill_q = []
    if do_qk:
        for mm in range(NTC):
            for p in range(NP):
                fill_q.append(("qk", p, 0, mm, pos[mm]))
                fill_q.append(("qk", p, 1, mm, pos[mm]))
    if do_v:
        for tt in range(NTT):
            fill_q.append(("v", tt, pos[tt // 4]))
    if do_proj:
        for tt in range(NTT):
            fill_q.append(("proj", tt, pos[tt // 4] + 1))
    fill_q.sort(key=lambda u: u[-1])

    cur_m = 0
    # pairs whose deferred norm tail has been emitted
    normb_done = [0 if do_norm else NP] * NTC
    normb_pending = []       # deferred (rb + multiply + otn-DMA) closures

    def flush_normb():
        while normb_pending:
            fn = normb_pending.pop(0)
            fn()

    # a few proj units are held back for the very last pair's scores loop,
    # which otherwise runs out of filler and lets the clock gate drop the
    # PE to half rate for the entire tail
    reserve = (
        {("proj", tt, pos[tt // 4] + 1) for tt in (9, 10, 11)}
        if do_proj else set()
    )
    allow_reserve = [False]

    def drain_one():
        for u in fill_q:
            if u in emitted:
                continue
            if u in reserve and not allow_reserve[0]:
                continue
            if u[0] == "proj" and normb_done[u[1] // 4] < NP:
                continue  # needs that chunk's norm fully emitted
            emit_unit(u)
            return True
        return False

    def warm(n):
        # dependency-free matmuls that keep the HAM clock gate at 8/8
        if do_attn and do_proj:
            for w in range(n):
                wk = ps_t.tile([128, 512], F32, tag="tt", name="warm")
                nc.tensor.matmul(wk, xt[0][:, 0:128], xt[1][:, 0:512],
                                 start=True, stop=True)

    # ---- PV deferral: pair p's P@V matmuls are emitted INSIDE pair p+1's
    # scores loop (their inputs are long since ready there), so they fill
    # the ACT-paced gaps instead of bunching after the loop.
    pv_pending = []      # per-j PV closures of the previous pair
    norm_a_pending = []  # previous pair's reciprocal chain, after its PV

    def flush_pv(k=None):
        n = len(pv_pending) if k is None else min(k, len(pv_pending))
        for _ in range(n):
            pv_pending.pop(0)()
        if not pv_pending:
            while norm_a_pending:
                norm_a_pending.pop(0)()

    def make_norm_a(p, m, otu_holder):
        def norm_a():
            # both heads have data at psum rows 0-63, denom at row 64
            # (symmetric ones-columns in vaug)
            otu_sb = small.tile([HS + 1, 2, 512], F32, tag="otusb",
                                name="otusb")
            nc.vector.tensor_copy(out=otu_sb, in_=otu_holder[0])
            # DVE reciprocal is ~8 cyc/elem along the free dim and the
            # denominators live on ONE partition ([1, 1024] -> 8.5us).
            # Repack them across 128 partitions with a pair of tiny
            # SBUF-SBUF DMAs so the reciprocal runs at FD=8 (~0.15us).
            dpack = small.tile([128, 8], F32, tag="dpack", name="dpack")
            nc.sync.dma_start(out=dpack, in_=otu_sb[HS:HS + 1, :, :])
            nc.vector.reciprocal(out=dpack, in_=dpack)
            # bf16 so the broadcast matmul is a plain bf16 matmul (fp32
            # matmuls run multi-pass at ~1us each)
            dpackb = small.tile([128, 8], BF16, tag="dpackb", name="dpackb")
            nc.vector.tensor_copy(out=dpackb, in_=dpack)
            rcpb = small.tile([1, 2, 512], BF16, tag="rcpb", name="rcpb")
            nc.sync.dma_start(out=rcpb, in_=dpackb)

            def normb():
                for e in range(2):
                    rb = ps_t.tile([HS, 512], F32, tag="tt", name="rb")
                    nc.tensor.matmul(
                        rb, ones_rows[0:1, 0:HS],
                        rcpb[0:1, e, :],
                        start=True, stop=True,
                    )
                    if e == 0:
                        nc.vector.tensor_mul(
                            out=otn[0:HS, p, m * 512:(m + 1) * 512],
                            in0=otu_sb[0:HS, 0, :],
                            in1=rb,
                        )
                    else:
                        otnorm = small.tile([HS, 512], BF16, tag="otnorm",
                                            name="otnorm")
                        nc.vector.tensor_mul(
                            out=otnorm,
                            in0=otu_sb[0:HS, 1, :],
                            in1=rb,
                        )
                        nc.sync.dma_start(
                            out=otn[HS:128, p, m * 512:(m + 1) * 512],
                            in_=otnorm,
                        )
                normb_done[m] += 1

            normb_pending.append(normb)

        return norm_a

    for m in order:
        cur_m = m
        jmax = 4 * m + 3
        for p in range(NP if (do_attn or do_norm) else 0):
            if do_qk:
                emit_unit(("qk", p, 0, m, pos[m]))   # Q chunk m of this pair
                for mm in range(m + 1):
                    emit_unit(("qk", p, 1, mm, pos[mm]))  # K chunks 0..m
            # lazily allocated by the first deferred PV matmul so the
            # single-buffer pool's reuse tracking sees accesses in order
            otu_holder = [None]
            if do_norm and not do_attn:
                otu_holder[0] = ps_otu.tile([HS + 1, 2, 512], F32, tag="otu",
                                            name="otu")
                nc.vector.memset(otu_holder[0], 1.0)
            pts = []
            for j in range((jmax + 1) if do_attn else 0):
                s0 = max(0, j - 4 * m)
                st = ps_st.tile([128, 2, 512], F32, tag="st", name="st")
                for e in range(2):
                    nc.tensor.matmul(
                        st[:, e, 128 * s0:512],
                        pairK[p][64 * e:64 * e + 64,
                                 j * 128:(j + 1) * 128],
                        pairQ[p][64 * e:64 * e + 64,
                                 m * 512 + 128 * s0:(m + 1) * 512],
                        start=True, stop=True,
                        tile_position=(64 * e, 0),
                    )
                pt = ptp.tile([128, 2, 512], BF16, tag="pt", name="pt")
                pts.append(pt)
                # one fused exp over both heads (2-bank strided AP)
                nc.scalar.activation(
                    out=pt[:, :, 128 * s0:512],
                    in_=st[:, :, 128 * s0:512],
                    func=mybir.ActivationFunctionType.Exp,
                    scale=SCALE,
                )
                if j >= 4 * m:
                    # zero the below-diagonal triangle of the diagonal
                    # subtile for both heads (keep where tq >= tk)
                    nc.gpsimd.affine_select(
                        out=pt[:, :, 128 * s0:128 * s0 + 128],
                        in_=pt[:, :, 128 * s0:128 * s0 + 128],
                        compare_op=mybir.AluOpType.is_ge,
                        fill=0.0, base=0,
                        pattern=[[0, 2], [1, 128]],
                        channel_multiplier=-1,
                    )
                flush_pv(1)  # one deferred PV matmul pair per score tile
                # fill the ACT-paced gap with proj units; drain faster in
                # the early chunks so the next chunk's inputs are ready
                # before its scores start (no bunching at the boundary)
                if m == NTC - 1 and p == NP - 1:
                    allow_reserve[0] = True
                    if not (drain_one() if j % 2 == 0 else False):
                        warm(1)
                elif j % 2 == 1 or m <= 1:
                    drain_one()
                # an older pair's deferred norm tail: its broadcast matmul
                # waits on the reciprocal DMA chain, which started only at
                # the END of the previous loop — flush a few tiles in (and
                # later still for the last pair, whose DVE queue is deeper)
                # so the wait is already satisfied and never blocks the PE
                if j == (6 if (m == NTC - 1 and p == NP - 1) else 3):
                    flush_normb()
            if do_v:
                for tt in range(4 * m + 4):
                    emit_unit(("v", tt, pos[tt // 4]))
            flush_pv()  # close out the previous pair entirely

            # the last pair's PV runs after its own scores loop, when the
            # st buffers are free: borrow one so the PV does not WAR-wait
            # on the previous pair's otu copy (ps_otu is single-buffered)
            last_pair = (m == order[-1] and p == NP - 1)

            def make_pv(p, m, j, jmax, otu_holder, pts, pool):
                def pv():
                    if otu_holder[0] is None:
                        otu_holder[0] = pool.tile(
                            [HS + 1, 2, 512], F32,
                            tag="st" if pool is ps_st else "otu", name="otu")
                    s0 = max(0, j - 4 * m)
                    for e in range(2):
                        nc.tensor.matmul(
                            otu_holder[0][:, e, 128 * s0:512],
                            vaug[:, j, p, 65 * e:65 * e + 65],
                            pts[j][:, e, 128 * s0:512],
                            start=(j == 0), stop=(j == jmax),
                            skip_group_check=True,
                        )
                return pv

            if do_attn:
                pool = ps_st if last_pair else ps_otu
                for j in range(jmax + 1):
                    pv_pending.append(
                        make_pv(p, m, j, jmax, otu_holder, pts, pool))
            if do_norm:
                norm_a_pending.append(make_norm_a(p, m, otu_holder))
            drain_one()  # one more filler unit at the pair boundary
            drain_one()
    # tail warm-keepers: the last pair's first PV waits on the previous
    # pair's otu copy (single-buffered psum), and the final norm chain
    # gates the last output projections — bridge both stalls
    warm(8)
    flush_pv()  # the last pair's P@V + its reciprocal chain
    cur_m = NTC
    tail_phase[0] = True
    warm(12)
    flush_normb()
    while drain_one():  # flush: the last chunk's output projections
        pass


_NC_CACHE = {}


def get_nc(repeat=1, phases=("v", "qk", "attn", "norm", "proj")):
    key = (repeat, tuple(phases))
    if key not in _NC_CACHE:
        nc = bacc.Bacc(
            "TRN2", target_bir_lowering=False, debug=False, num_devices=8
        )
        _NC_CACHE[key] = build_kernel(nc, repeat=repeat, phases=phases)
    return _NC_CACHE[key]


def make_in_maps(x, Wq, Wk, Wv, Wp):
    x = np.asarray(x, dtype=np.float32)
    Wq = np.asarray(Wq, dtype=np.float32)
    Wk = np.asarray(Wk, dtype=np.float32)
    Wv = np.asarray(Wv, dtype=np.float32)
    Wp = np.asarray(Wp, dtype=np.float32)
    bf = ml_dtypes.bfloat16
    in_maps = []
    for c in range(8):
        b = c // 2
        hs = HL * (c % 2)
        xT = np.ascontiguousarray(x[b].T).astype(bf)
        # wqkp: pair-stacked Q/K weights, partition-major for one-DMA loads:
        # [NP, 2, 128(row), NCT, 128]; the last 128 columns are
        # [head 2p (64) | head 2p+1 (64)] of Wq (qk=0) or Wk.
        wqkp = np.empty((NP, 2, 128, NCT, 128), dtype=bf)
        for p in range(NP):
            for qk, W in enumerate((Wq, Wk)):
                stacked = np.concatenate(
                    [W[hs + 2 * p], W[hs + 2 * p + 1]], axis=1)  # [C, 128]
                wqkp[p, qk] = stacked.reshape(NCT, 128, 128).transpose(
                    1, 0, 2).astype(bf)
        # wv: [128(row), NCT, 384], cols = [V_h0 V_h2 V_h4 | V_h1 V_h3 V_h5]
        order = [0, 2, 4, 1, 3, 5]
        wv_full = np.concatenate(
            [Wv[hs + h] for h in order], axis=1)  # [C, 384]
        wv = np.ascontiguousarray(
            wv_full.reshape(NCT, 128, HL * HS).transpose(1, 0, 2)
        ).astype(bf)
        # wpt: Wp[:, i_slice].T -> [384, C] -> [128(row), 3, C]
        wpt = np.ascontiguousarray(
            Wp[:, hs * HS:(hs + HL) * HS].T.reshape(3, 128, C).transpose(
                1, 0, 2)
        ).astype(bf)
        in_maps.append({"xT": xT, "wqkp": wqkp, "wv": wv, "wpt": wpt})
    return in_maps


def run(x, Wq, Wk, Wv, Wp, bp, trace=False):
    nc = get_nc()
    in_maps = make_in_maps(x, Wq, Wk, Wv, Wp)
    res = bass_utils.run_bass_kernel_spmd(
        nc, in_maps, core_ids=list(range(8)), trace=trace
    )
    y = np.zeros((B, T, C), dtype=np.float32)
    for c in range(8):
        y[c // 2] += res.results[c]["y"].astype(np.float32)
    y += np.asarray(bp, dtype=np.float32)
    return y, res


def kernel(x, Wq, Wk, Wv, Wp, bp):
    y, _ = run(x, Wq, Wk, Wv, Wp, bp)
    return y


def make_runner(nc):
    """Build the sharded PJRT callable once (mirrors the tail of
    bass2jax.run_bass_via_pjrt) so repeated timed executions don't re-trace.
    Returns (fn, prep) where prep(in_maps) device_puts the inputs and
    fn(device_inputs) -> per-core output dicts (blocking)."""
    import jax
    from jax.experimental.shard_map import shard_map
    from jax.sharding import Mesh, PartitionSpec, NamedSharding
    from concourse import mybir as _mybir
    from concourse.bass2jax import (
        _bass_exec_p, install_neuronx_cc_hook, partition_id_tensor,
    )

    install_neuronx_cc_hook()
    n_cores = 8
    partition_name = (
        nc.partition_id_tensor.name if nc.partition_id_tensor else None
    )
    in_names, out_names, out_avals = [], [], []
    for alloc in nc.m.functions[0].allocations:
        if not isinstance(alloc, _mybir.MemoryLocationSet):
            continue
        name = alloc.memorylocations[0].name
        if alloc.kind == "ExternalInput":
            if name != partition_name:
                in_names.append(name)
        elif alloc.kind == "ExternalOutput":
            out_names.append(name)
            out_avals.append(
                jax.core.ShapedArray(
                    tuple(alloc.tensor_shape), _mybir.dt.np(alloc.dtype)
                )
            )
    n_params = len(in_names)
    n_outs = len(out_avals)
    all_in_names = in_names + out_names
    if partition_name is not None:
        all_in_names.append(partition_name)

    def _body(*args):
        operands = list(args)
        if partition_name is not None:
            operands.append(partition_id_tensor())
        outs = _bass_exec_p.bind(
            *operands,
            out_avals=tuple(out_avals),
            in_names=tuple(all_in_names),
            out_names=tuple(out_names),
            lowering_input_output_aliases=(),
            sim_require_finite=True,
            sim_require_nnan=True,
            nc=nc,
        )
        return tuple(outs)

    devices = jax.devices()[:n_cores]
    mesh = Mesh(np.array(devices), ("core",))
    sharded = jax.jit(
        shard_map(
            _body, mesh=mesh,
            in_specs=(PartitionSpec("core"),) * (n_params + n_outs),
            out_specs=(PartitionSpec("core"),) * n_outs,
            check_rep=False,
        ),
        donate_argnums=tuple(range(n_params, n_params + n_outs)),
        keep_unused=True,
    )
    shd = NamedSharding(mesh, PartitionSpec("core"))

    def prep(in_maps):
        return [
            jax.device_put(
                np.concatenate([in_maps[c][nm] for c in range(n_cores)], axis=0),
                shd,
            )
            for nm in in_names
        ]

    def zeros():
        return [
            jax.device_put(
                np.zeros((n_cores * a.shape[0], *a.shape[1:]), a.dtype), shd
            )
            for a in out_avals
        ]

    def fn(dev_inputs, dev_zeros):
        outs = sharded(*dev_inputs, *dev_zeros)
        jax.block_until_ready(outs)
        return outs

    def make_loop_fn(n_iters):
        def _body_n(*args):
            ins = args[:n_params]
            carry = tuple(args[n_params:])

            def step(i, carry):
                operands = list(ins) + list(carry)
                if partition_name is not None:
                    operands.append(partition_id_tensor())
                outs = _bass_exec_p.bind(
                    *operands,
                    out_avals=tuple(out_avals),
                    in_names=tuple(all_in_names),
                    out_names=tuple(out_names),
                    lowering_input_output_aliases=(),
                    sim_require_finite=True,
                    sim_require_nnan=True,
                    nc=nc,
                )
                return tuple(outs)

            return jax.lax.fori_loop(0, n_iters, step, carry)

        looped = jax.jit(
            shard_map(
                _body_n, mesh=mesh,
                in_specs=(PartitionSpec("core"),) * (n_params + n_outs),
                out_specs=(PartitionSpec("core"),) * n_outs,
                check_rep=False,
            ),
            donate_argnums=tuple(range(n_params, n_params + n_outs)),
            keep_unused=True,
        )

        def run_n(dev_inputs, dev_zeros):
            outs = looped(*dev_inputs, *dev_zeros)
            jax.block_until_ready(outs)
            return outs

        return run_n

    return fn, prep, zeros, out_names, make_loop_fn
